# revision 32
# baseline (speedup 1.0000x reference)
"""Multi-head attention TRN2 kernel (v2, bf16 software-pipelined).

Sharding: 8 cores = 4 batches x 2 head-groups (Megatron tensor parallel over
the 16 heads: Wq/Wk/Wv column-sharded, Wo row-sharded; partial outputs summed
per batch on the host).

Per-core schedule (batch b, head-group g -> 8 local heads, 4 head-pairs c):
  prologue: kT(c=0) + qT(st=0,c=0) projections; v/kT(c>0)/qT interleave later
  main loop over (st, c): 16 t-chunks:
     scoresT[t,s] pair = kT_h.T @ qT_h      (K=64, j-pair co-executes on PE)
     ex = exp(scores/8) on ACT (PSUM->SBUF bf16)
     cx[65,s] += v_aug.T @ ex               (ones column -> softmax sums)
  producer mms (v proj, later kT/qT, out-proj) pumped into PE slack so the
  ACT engine (exp, ~283us total) stays saturated; normalize on DVE/Pool.
  out partial = ctxT.T @ WoT accumulated per 128-row chunk, DMA'd out.
"""

import os
import sys
from collections import deque
from contextlib import ExitStack

for _p in ("/opt/trn_rl_repo", "/root/.axon_site/_ro/trn_rl_repo"):
    if os.path.isdir(_p) and _p not in sys.path:
        sys.path.insert(0, _p)
        break

import numpy as np

import concourse.bass as bass
import concourse.bacc as bacc
import concourse.mybir as mybir
import concourse.tile as tile

B, S, E, H, D = 4, 2048, 1024, 16, 64
HG = 2          # head groups (tensor-parallel factor)
DH = E // HG    # 512 dims per head group (8 heads)
HPG = H // HG   # 8 heads per group
NCORES = B * HG

EC = E // 128   # 8 contraction chunks for projections
DC = DH // 128  # 4 d-chunks (head pairs)
TC = S // 128   # 16 t chunks
ST = S // 512   # 4 s tiles
SQ = S // 512   # 4 column blocks for projections
DA = D + 1      # 65: head dim + ones column

F32 = mybir.dt.float32
MM_DT = mybir.dt.bfloat16
SCALE = 1.0 / np.sqrt(D)


def build_nc():
    nc = bacc.Bacc()
    xqT = nc.declare_dram_parameter("xqT", [E, S], MM_DT, isOutput=False)
    xkT = nc.declare_dram_parameter("xkT", [E, S], MM_DT, isOutput=False)
    xvT = nc.declare_dram_parameter("xvT", [E, S], MM_DT, isOutput=False)
    wqT = nc.declare_dram_parameter("wqT", [E, DH], MM_DT, isOutput=False)
    wkT = nc.declare_dram_parameter("wkT", [E, DH], MM_DT, isOutput=False)
    wvT = nc.declare_dram_parameter("wvT", [E, DH], MM_DT, isOutput=False)
    woT = nc.declare_dram_parameter("woT", [DH, E], MM_DT, isOutput=False)
    out = nc.declare_dram_parameter("out", [S, E], F32, isOutput=True)

    with (
        nc.allow_low_precision(reason="bf16 matmul operands"),
        tile.TileContext(nc) as tc,
        ExitStack() as ctx,
    ):
        _emit(ctx, tc, xqT, xkT, xvT, wqT, wkT, wvT, woT, out)
    nc.compile()
    return nc


def _emit(ctx, tc, xqT, xkT, xvT, wqT, wkT, wvT, woT, out):
    nc = tc.nc

    big = ctx.enter_context(tc.tile_pool(name="big", bufs=1))
    # qT/kT/ctxT: [p, c, s] with local dim ld = 128*c + p
    # (head 2c on partitions 0-63, head 2c+1 on 64-127)
    qT_sb = big.tile([128, DC, S], MM_DT, tag="qT")
    kT_sb = big.tile([128, DC, S], MM_DT, tag="kT")
    ctxT_sb = big.tile([128, DC, S], MM_DT, tag="ctx")
    # v_aug: [t%128, t_chunk, head, 65]; col 64 is the ones column
    v_sb = big.tile([128, TC, HPG, DA], MM_DT, tag="v")
    wq_sb = big.tile([128, EC, DH], MM_DT, tag="wq")
    wk_sb = big.tile([128, EC, DH], MM_DT, tag="wk")
    wv_sb = big.tile([128, EC, DH], MM_DT, tag="wv")
    wo_sb = big.tile([128, DC, E], MM_DT, tag="wo")
    xk_st = big.tile([128, EC, S], MM_DT, tag="xk")
    xv_st = big.tile([128, EC, S], MM_DT, tag="xv")
    ones = big.tile([128, 1], MM_DT, tag="ones")
    nc.vector.memset(ones, 1.0)
    nc.vector.tensor_copy(
        v_sb[:, :, :, D : D + 1],
        ones.to_broadcast((128, TC * HPG)).rearrange(
            "p (t h o) -> p t h o", t=TC, h=HPG
        ),
    )

    xqp = ctx.enter_context(tc.tile_pool(name="xqp", bufs=2))
    expool = ctx.enter_context(tc.tile_pool(name="ex", bufs=5))
    osb = ctx.enter_context(tc.tile_pool(name="osb", bufs=2))
    small = ctx.enter_context(tc.tile_pool(name="small", bufs=2))
    scps = ctx.enter_context(tc.tile_pool(name="scps", bufs=2, space="PSUM"))
    cxps = ctx.enter_context(tc.tile_pool(name="cxps", bufs=2, space="PSUM"))
    accps = ctx.enter_context(tc.tile_pool(name="accps", bufs=2, space="PSUM"))

    # ---- DMA helpers: alternate the two DMA-capable idle engines ----------
    dma_state = [0]

    def dma(out_ap, in_ap):
        eng = nc.sync if dma_state[0] % 2 == 0 else nc.gpsimd
        dma_state[0] += 1
        eng.dma_start(out=out_ap, in_=in_ap)

    # priority-ordered loads: what the prologue needs first
    for e in range(EC):
        dma(wk_sb[:, e, :], wkT[128 * e : 128 * (e + 1), :])
    for e in range(EC):
        dma(xk_st[:, e, 0:512], xkT[128 * e : 128 * (e + 1), 0:512])
    for e in range(EC):
        dma(wq_sb[:, e, :], wqT[128 * e : 128 * (e + 1), :])
    xq_tiles = {}
    xq_tiles[0] = xqp.tile([128, EC, 512], MM_DT, tag="xq", name="xq_st0")
    for e in range(EC):
        dma(xq_tiles[0][:, e, :], xqT[128 * e : 128 * (e + 1), 0:512])
    # interleave the remaining xk blocks with wv/xv so both the kT and the v
    # producer chains get fed in parallel
    for e in range(EC):
        dma(
            xk_st[:, e, 512 : 1024],
            xkT[128 * e : 128 * (e + 1), 512 : 1024],
        )
    for e in range(EC):
        dma(wv_sb[:, e, :], wvT[128 * e : 128 * (e + 1), :])
    for sq, xsq in ((0, 2), (1, 3)):
        for e in range(EC):
            dma(
                xv_st[:, e, 512 * sq : 512 * (sq + 1)],
                xvT[128 * e : 128 * (e + 1), 512 * sq : 512 * (sq + 1)],
            )
        for e in range(EC):
            dma(
                xk_st[:, e, 512 * xsq : 512 * (xsq + 1)],
                xkT[128 * e : 128 * (e + 1), 512 * xsq : 512 * (xsq + 1)],
            )
    for sq in range(2, SQ):
        for e in range(EC):
            dma(
                xv_st[:, e, 512 * sq : 512 * (sq + 1)],
                xvT[128 * e : 128 * (e + 1), 512 * sq : 512 * (sq + 1)],
            )
    for a in range(DC):
        dma(wo_sb[:, a, :], woT[128 * a : 128 * (a + 1), :])

    # ---- producer generators (yield after each matmul) --------------------
    def kT_gen(c, sq):
        acc = accps.tile([128, 512], F32, tag="acc", name=f"kacc_{c}_{sq}")
        for e in range(EC):
            nc.tensor.matmul(
                acc,
                lhsT=wk_sb[:, e, 128 * c : 128 * (c + 1)],
                rhs=xk_st[:, e, 512 * sq : 512 * (sq + 1)],
                start=(e == 0),
                stop=(e == EC - 1),
            )
            yield
        nc.vector.tensor_copy(kT_sb[:, c, 512 * sq : 512 * (sq + 1)], acc)

    def v_gen(tt):
        acc = accps.tile([128, 512], F32, tag="acc", name=f"vacc_{tt}")
        for e in range(EC):
            nc.tensor.matmul(
                acc,
                lhsT=xv_st[:, e, 128 * tt : 128 * (tt + 1)],
                rhs=wv_sb[:, e, :],
                start=(e == 0),
                stop=(e == EC - 1),
            )
            yield
        nc.vector.tensor_copy(
            v_sb[:, tt, :, 0:D], acc.rearrange("p (h d) -> p h d", h=HPG)
        )

    def qT_gen(st, c):
        xq = xq_tiles[st]
        acc = accps.tile([128, 512], F32, tag="acc", name=f"qacc_{st}_{c}")
        for e in range(EC):
            nc.tensor.matmul(
                acc,
                lhsT=wq_sb[:, e, 128 * c : 128 * (c + 1)],
                rhs=xq[:, e, :],
                start=(e == 0),
                stop=(e == EC - 1),
            )
            yield
        nc.vector.tensor_copy(qT_sb[:, c, 512 * st : 512 * (st + 1)], acc)

    def fp_gen(st, si, o_tile):
        r0 = 512 * st + 128 * si
        for et in range(2):
            fp = accps.tile([128, 512], F32, tag="acc", name=f"fp_{r0}_{et}")
            for cc in range(DC):
                nc.tensor.matmul(
                    fp,
                    lhsT=ctxT_sb[:, cc, r0 : r0 + 128],
                    rhs=wo_sb[:, cc, 512 * et : 512 * (et + 1)],
                    start=(cc == 0),
                    stop=(cc == DC - 1),
                )
                yield
            nc.vector.tensor_copy(o_tile[:, 512 * et : 512 * (et + 1)], fp)
        for p0 in range(0, 128, 32):
            dma(out[r0 + p0 : r0 + p0 + 32, :], o_tile[p0 : p0 + 32, :])

    # producer queue machinery: (key, generator) FIFO with forced drains
    producers = deque()
    done_keys = set()
    cur = [None, None]  # key, generator

    def _finish_cur():
        done_keys.add(cur[0])
        cur[0] = cur[1] = None

    def pump(n):
        emitted = 0
        while emitted < n:
            if cur[1] is None:
                if not producers:
                    return
                cur[0], cur[1] = producers.popleft()
            try:
                next(cur[1])
                emitted += 1
            except StopIteration:
                _finish_cur()

    def pump_until(key):
        while key not in done_keys:
            if cur[1] is None:
                if not producers:
                    raise RuntimeError(f"producer underflow waiting for {key}")
                cur[0], cur[1] = producers.popleft()
            try:
                while True:
                    next(cur[1])
            except StopIteration:
                _finish_cur()

    # ---- prologue: just enough for the first scores matmul ---------------
    for _ in kT_gen(0, 0):
        pass
    for _ in qT_gen(0, 0):
        pass
    done_keys.add(("kT", 0, 0))
    done_keys.add(("qT", 0, 0))

    # initial producer order: v first (needed by cxmm), then kT/qT for the
    # upcoming head-pairs / s-tiles
    for sq in range(1, SQ):
        producers.append((("kT", 0, sq), kT_gen(0, sq)))
    for tt in range(8):
        producers.append((("v", tt), v_gen(tt)))
    producers.append((("qT", 0, 1), qT_gen(0, 1)))
    for sq in range(2):
        producers.append((("kT", 1, sq), kT_gen(1, sq)))
    for tt in range(8, TC):
        producers.append((("v", tt), v_gen(tt)))
    for sq in range(2, SQ):
        producers.append((("kT", 1, sq), kT_gen(1, sq)))
    producers.append((("qT", 0, 2), qT_gen(0, 2)))
    for sq in range(SQ):
        producers.append((("kT", 2, sq), kT_gen(2, sq)))
    producers.append((("qT", 0, 3), qT_gen(0, 3)))
    for sq in range(SQ):
        producers.append((("kT", 3, sq), kT_gen(3, sq)))

    # ---- main attention loop ---------------------------------------------
    # deferred normalize: the previous window's recip/broadcast/mul runs as
    # six small steps spread across the current window, so no single DVE
    # insertion blocks producer PSUM->SBUF copies for long.
    norm_steps = deque()

    def run_norm_step():
        if norm_steps:
            norm_steps.popleft()()

    def queue_norm(st, c, cxs):
        s0 = 512 * st
        recs = [None, None]
        bcs = [None, None]

        def recip_half(j, h):
            def fn():
                if h == 0:
                    recs[j] = small.tile(
                        [1, 512], F32, tag="rec", name=f"rec{st}_{c}_{j}"
                    )
                nc.vector.reciprocal(
                    recs[j][:, 256 * h : 256 * (h + 1)],
                    cxs[j][D : D + 1, 256 * h : 256 * (h + 1)],
                )

            return fn

        def bcast(j):
            def fn():
                bcs[j] = small.tile(
                    [64, 512], F32, tag="tmp", name=f"bc{st}_{c}_{j}"
                )
                nc.gpsimd.partition_broadcast(bcs[j], recs[j])

            return fn

        def mul(j):
            def fn():
                nc.vector.tensor_mul(
                    ctxT_sb[64 * j : 64 * (j + 1), c, s0 : s0 + 512],
                    cxs[j][0:D, :],
                    bcs[j],
                )
                # whole s-tile normalized -> its output projection may run
                if j == 1 and c == DC - 1:
                    for si in range(4):
                        o_tile = osb.tile(
                            [128, E], F32, tag="osb", name=f"osb_{st}_{si}"
                        )
                        producers.append((("fp", st, si), fp_gen(st, si, o_tile)))

            return fn

        norm_steps.append(recip_half(0, 0))
        norm_steps.append(recip_half(0, 1))
        norm_steps.append(bcast(0))
        norm_steps.append(recip_half(1, 0))
        norm_steps.append(recip_half(1, 1))

        def mul0_then(fn2):
            def fn():
                mul(0)()
                fn2()

            return fn

        norm_steps.append(mul0_then(bcast(1)))
        norm_steps.append(mul(1))

    for st in range(ST):
        s0 = 512 * st
        # stage the next s-tile's xq chunks well before qT(st+1) producers run
        if st + 1 < ST:
            nxt = xqp.tile([128, EC, 512], MM_DT, tag="xq", name=f"xq_st{st+1}")
            xq_tiles[st + 1] = nxt
            for e in range(EC):
                dma(
                    nxt[:, e, :],
                    xqT[128 * e : 128 * (e + 1), 512 * (st + 1) : 512 * (st + 2)],
                )
        for c in range(DC):
            if not (st == 0 and c == 0):
                pump_until(("qT", st, c))
            cx = [
                cxps.tile([DA, 512], F32, tag="cx", name=f"cx{st}_{c}_{j}")
                for j in range(2)
            ]
            first = st == 0 and c == 0
            ex_tiles = {}

            def emit_sc(t):
                if not (st == 0 and c == 0 and t < 4):
                    pump_until(("kT", c, t // 4))
                sc = scps.tile([128, 1024], F32, tag="sc")
                for j in range(2):
                    nc.tensor.matmul(
                        sc[:, 512 * j : 512 * (j + 1)],
                        lhsT=kT_sb[64 * j : 64 * (j + 1), c, 128 * t : 128 * (t + 1)],
                        rhs=qT_sb[64 * j : 64 * (j + 1), c, s0 : s0 + 512],
                        start=True,
                        stop=True,
                    )
                ex = expool.tile([128, 1024], MM_DT, tag="ex")
                nc.scalar.activation(
                    out=ex,
                    in_=sc,
                    func=mybir.ActivationFunctionType.Exp,
                    scale=float(SCALE),
                )
                ex_tiles[t] = ex

            def emit_cx(t):
                pump_until(("v", t))
                ex = ex_tiles.pop(t)
                for j in range(2):
                    nc.tensor.matmul(
                        cx[j],
                        lhsT=v_sb[:, t, 2 * c + j, :],
                        rhs=ex[:, 512 * j : 512 * (j + 1)],
                        start=(t == 0),
                        stop=(t == TC - 1),
                    )

            if first:
                # sc/exp stream ahead while v is still being produced
                for t in range(TC):
                    emit_sc(t)
                    pump(4)
                for t in range(TC):
                    emit_cx(t)
                    pump(1)
                    if t >= 3:
                        run_norm_step()
            else:
                for t in range(TC):
                    emit_sc(t)
                    emit_cx(t)
                    pump(2)
                    if t >= 3:
                        run_norm_step()

            # free the cx PSUM bank fast; the rest of the normalize is
            # deferred into the next window
            cxs = []
            for j in range(2):
                t_ = small.tile([DA, 512], F32, tag="cxs", name=f"cxs{st}_{c}_{j}")
                nc.vector.tensor_copy(t_, cx[j])
                cxs.append(t_)
            queue_norm(st, c, cxs)

        if st + 1 < ST:
            for c2 in range(DC):
                producers.append((("qT", st + 1, c2), qT_gen(st + 1, c2)))

    # ---- epilogue: last normalize + drain remaining producers -------------
    while norm_steps:
        run_norm_step()
    pump(10**9)


_BUILT = {}


def _get_nc():
    if "nc" not in _BUILT:
        _BUILT["nc"] = build_nc()
    return _BUILT["nc"]


def make_in_maps(query, key, value, Wq, Wk, Wv, Wo):
    ndt = mybir.dt.np(MM_DT)
    query = np.asarray(query, np.float32).astype(ndt)
    key = np.asarray(key, np.float32).astype(ndt)
    value = np.asarray(value, np.float32).astype(ndt)
    Wq = np.asarray(Wq, np.float32).astype(ndt)
    Wk = np.asarray(Wk, np.float32).astype(ndt)
    Wv = np.asarray(Wv, np.float32).astype(ndt)
    Wo = np.asarray(Wo, np.float32).astype(ndt)

    xqT = [np.ascontiguousarray(query[b].T) for b in range(B)]
    xkT = [np.ascontiguousarray(key[b].T) for b in range(B)]
    xvT = [np.ascontiguousarray(value[b].T) for b in range(B)]
    wqT = [np.ascontiguousarray(Wq[DH * g : DH * (g + 1), :].T) for g in range(HG)]
    wkT = [np.ascontiguousarray(Wk[DH * g : DH * (g + 1), :].T) for g in range(HG)]
    wvT = [np.ascontiguousarray(Wv[DH * g : DH * (g + 1), :].T) for g in range(HG)]
    woT = [np.ascontiguousarray(Wo[:, DH * g : DH * (g + 1)].T) for g in range(HG)]

    in_maps = []
    for core in range(NCORES):
        b, g = core // HG, core % HG
        in_maps.append(
            {
                "xqT": xqT[b],
                "xkT": xkT[b],
                "xvT": xvT[b],
                "wqT": wqT[g],
                "wkT": wkT[g],
                "wvT": wvT[g],
                "woT": woT[g],
            }
        )
    return in_maps


def assemble(core_outs):
    out = np.empty((B, S, E), np.float32)
    for b in range(B):
        out[b] = core_outs[HG * b]
        for g in range(1, HG):
            out[b] += core_outs[HG * b + g]
    return out


def kernel(query, key, value, Wq, Wk, Wv, Wo):
    from concourse.bass_utils import run_bass_kernel_spmd

    nc = _get_nc()
    in_maps = make_in_maps(query, key, value, Wq, Wk, Wv, Wo)
    res = run_bass_kernel_spmd(nc, in_maps, list(range(NCORES)))
    return assemble([r["out"] for r in res.results])


# revision 42
# speedup vs baseline: 1.1620x; 1.1620x over previous
"""Multi-head attention TRN2 kernel (v2, bf16 software-pipelined).

Sharding: 8 cores = 4 batches x 2 head-groups (Megatron tensor parallel over
the 16 heads: Wq/Wk/Wv column-sharded, Wo row-sharded; partial outputs summed
per batch on the host).

Per-core schedule (batch b, head-group g -> 8 local heads, 4 head-pairs c):
  prologue: kT(c=0) + qT(st=0,c=0) projections; v/kT(c>0)/qT interleave later
  main loop over (st, c): 16 t-chunks:
     scoresT[t,s] pair = kT_h.T @ qT_h      (K=64, j-pair co-executes on PE)
     ex = exp(scores/8) on ACT (PSUM->SBUF bf16)
     cx[65,s] += v_aug.T @ ex               (ones column -> softmax sums)
  producer mms (v proj, later kT/qT, out-proj) pumped into PE slack so the
  ACT engine (exp, ~283us total) stays saturated; normalize on DVE/Pool.
  out partial = ctxT.T @ WoT accumulated per 128-row chunk, DMA'd out.
"""

import os
import sys
from collections import deque
from contextlib import ExitStack

for _p in ("/opt/trn_rl_repo", "/root/.axon_site/_ro/trn_rl_repo"):
    if os.path.isdir(_p) and _p not in sys.path:
        sys.path.insert(0, _p)
        break

import numpy as np

import concourse.bass as bass
import concourse.bacc as bacc
import concourse.mybir as mybir
import concourse.tile as tile

B, S, E, H, D = 4, 2048, 1024, 16, 64
HG = 2          # head groups (tensor-parallel factor)
DH = E // HG    # 512 dims per head group (8 heads)
HPG = H // HG   # 8 heads per group
NCORES = B * HG

EC = E // 128   # 8 contraction chunks for projections
DC = DH // 128  # 4 d-chunks (head pairs)
TC = S // 128   # 16 t chunks
ST = S // 512   # 4 s tiles
SQ = S // 512   # 4 column blocks for projections
DA = D + 1      # 65: head dim + ones column

F32 = mybir.dt.float32
MM_DT = mybir.dt.bfloat16
SCALE = 1.0 / np.sqrt(D)


def build_nc():
    nc = bacc.Bacc()
    xqT = nc.declare_dram_parameter("xqT", [E, S], MM_DT, isOutput=False)
    xkT = nc.declare_dram_parameter("xkT", [E, S], MM_DT, isOutput=False)
    xvT = nc.declare_dram_parameter("xvT", [E, S], MM_DT, isOutput=False)
    wqT = nc.declare_dram_parameter("wqT", [E, DH], MM_DT, isOutput=False)
    wkT = nc.declare_dram_parameter("wkT", [E, DH], MM_DT, isOutput=False)
    wvT = nc.declare_dram_parameter("wvT", [E, DH], MM_DT, isOutput=False)
    woT = nc.declare_dram_parameter("woT", [DH, E], MM_DT, isOutput=False)
    out = nc.declare_dram_parameter("out", [S, E], F32, isOutput=True)

    with (
        nc.allow_low_precision(reason="bf16 matmul operands"),
        tile.TileContext(nc) as tc,
        ExitStack() as ctx,
    ):
        _emit(ctx, tc, xqT, xkT, xvT, wqT, wkT, wvT, woT, out)
    nc.compile()
    return nc


def _emit(ctx, tc, xqT, xkT, xvT, wqT, wkT, wvT, woT, out):
    nc = tc.nc

    big = ctx.enter_context(tc.tile_pool(name="big", bufs=1))
    # qT/kT/ctxT: [p, c, s] with local dim ld = 128*c + p
    # (head 2c on partitions 0-63, head 2c+1 on 64-127)
    qT_sb = big.tile([128, DC, S], MM_DT, tag="qT")
    kT_sb = big.tile([128, DC, S], MM_DT, tag="kT")
    ctxT_sb = big.tile([128, DC, S], MM_DT, tag="ctx")
    # v_aug: [t%128, t_chunk, head, 65]; col 64 is the ones column
    v_sb = big.tile([128, TC, HPG, DA], MM_DT, tag="v")
    wq_sb = big.tile([128, EC, DH], MM_DT, tag="wq")
    wk_sb = big.tile([128, EC, DH], MM_DT, tag="wk")
    wv_sb = big.tile([128, EC, DH], MM_DT, tag="wv")
    wo_sb = big.tile([128, DC, E], MM_DT, tag="wo")
    xk_st = big.tile([128, EC, S], MM_DT, tag="xk")
    xv_st = big.tile([128, EC, S], MM_DT, tag="xv")
    ones = big.tile([128, 1], MM_DT, tag="ones")
    nc.vector.memset(ones, 1.0)

    nc.vector.tensor_copy(
        v_sb[:, :, :, D : D + 1],
        ones.to_broadcast((128, TC * HPG)).rearrange(
            "p (t h o) -> p t h o", t=TC, h=HPG
        ),
    )

    xqp = ctx.enter_context(tc.tile_pool(name="xqp", bufs=2))
    expool = ctx.enter_context(tc.tile_pool(name="ex", bufs=5))
    osb = ctx.enter_context(tc.tile_pool(name="osb", bufs=2))
    small = ctx.enter_context(tc.tile_pool(name="small", bufs=2))
    scps = ctx.enter_context(tc.tile_pool(name="scps", bufs=2, space="PSUM"))
    cxps = ctx.enter_context(tc.tile_pool(name="cxps", bufs=2, space="PSUM"))
    accps = ctx.enter_context(tc.tile_pool(name="accps", bufs=2, space="PSUM"))

    # ---- DMA helpers: alternate the two DMA-capable idle engines ----------
    dma_state = [0]

    def dma(out_ap, in_ap):
        eng = nc.sync if dma_state[0] % 2 == 0 else nc.gpsimd
        dma_state[0] += 1
        eng.dma_start(out=out_ap, in_=in_ap)

    # priority-ordered loads: what the prologue needs first
    for e in range(EC):
        dma(wk_sb[:, e, :], wkT[128 * e : 128 * (e + 1), :])
    for e in range(EC):
        dma(xk_st[:, e, 0:512], xkT[128 * e : 128 * (e + 1), 0:512])
    for e in range(EC):
        dma(wq_sb[:, e, :], wqT[128 * e : 128 * (e + 1), :])
    xq_tiles = {}
    xq_tiles[0] = xqp.tile([128, EC, 512], MM_DT, tag="xq", name="xq_st0")
    for e in range(EC):
        dma(xq_tiles[0][:, e, :], xqT[128 * e : 128 * (e + 1), 0:512])
    # interleave the remaining xk blocks with wv/xv so both the kT and the v
    # producer chains get fed in parallel
    for e in range(EC):
        dma(
            xk_st[:, e, 512 : 1024],
            xkT[128 * e : 128 * (e + 1), 512 : 1024],
        )
    for e in range(EC):
        dma(wv_sb[:, e, :], wvT[128 * e : 128 * (e + 1), :])
    for sq, xsq in ((0, 2), (1, 3)):
        for e in range(EC):
            dma(
                xv_st[:, e, 512 * sq : 512 * (sq + 1)],
                xvT[128 * e : 128 * (e + 1), 512 * sq : 512 * (sq + 1)],
            )
        for e in range(EC):
            dma(
                xk_st[:, e, 512 * xsq : 512 * (xsq + 1)],
                xkT[128 * e : 128 * (e + 1), 512 * xsq : 512 * (xsq + 1)],
            )
    for sq in range(2, SQ):
        for e in range(EC):
            dma(
                xv_st[:, e, 512 * sq : 512 * (sq + 1)],
                xvT[128 * e : 128 * (e + 1), 512 * sq : 512 * (sq + 1)],
            )
    for a in range(DC):
        dma(wo_sb[:, a, :], woT[128 * a : 128 * (a + 1), :])

    # ---- producer generators (yield after each matmul) --------------------
    def kT_gen(c, sq):
        acc = accps.tile([128, 512], F32, tag="acc", name=f"kacc_{c}_{sq}")
        for e in range(EC):
            nc.tensor.matmul(
                acc,
                lhsT=wk_sb[:, e, 128 * c : 128 * (c + 1)],
                rhs=xk_st[:, e, 512 * sq : 512 * (sq + 1)],
                start=(e == 0),
                stop=(e == EC - 1),
            )
            yield
        nc.vector.tensor_copy(kT_sb[:, c, 512 * sq : 512 * (sq + 1)], acc)

    def v_gen(tt):
        acc = accps.tile([128, 512], F32, tag="acc", name=f"vacc_{tt}")
        for e in range(EC):
            nc.tensor.matmul(
                acc,
                lhsT=xv_st[:, e, 128 * tt : 128 * (tt + 1)],
                rhs=wv_sb[:, e, :],
                start=(e == 0),
                stop=(e == EC - 1),
            )
            yield
        nc.vector.tensor_copy(
            v_sb[:, tt, :, 0:D], acc.rearrange("p (h d) -> p h d", h=HPG)
        )

    def qT_gen(st, c):
        xq = xq_tiles[st]
        acc = accps.tile([128, 512], F32, tag="acc", name=f"qacc_{st}_{c}")
        for e in range(EC):
            nc.tensor.matmul(
                acc,
                lhsT=wq_sb[:, e, 128 * c : 128 * (c + 1)],
                rhs=xq[:, e, :],
                start=(e == 0),
                stop=(e == EC - 1),
            )
            yield
        nc.vector.tensor_copy(qT_sb[:, c, 512 * st : 512 * (st + 1)], acc)

    def fp_gen(st, si, o_tile):
        r0 = 512 * st + 128 * si
        for et in range(2):
            fp = accps.tile([128, 512], F32, tag="acc", name=f"fp_{r0}_{et}")
            for cc in range(DC):
                nc.tensor.matmul(
                    fp,
                    lhsT=ctxT_sb[:, cc, r0 : r0 + 128],
                    rhs=wo_sb[:, cc, 512 * et : 512 * (et + 1)],
                    start=(cc == 0),
                    stop=(cc == DC - 1),
                )
                yield
            nc.vector.tensor_copy(o_tile[:, 512 * et : 512 * (et + 1)], fp)
        for p0 in range(0, 128, 32):
            dma(out[r0 + p0 : r0 + p0 + 32, :], o_tile[p0 : p0 + 32, :])

    # producer queue machinery: (key, generator) FIFO with forced drains
    producers = deque()
    done_keys = set()
    cur = [None, None]  # key, generator

    def _finish_cur():
        done_keys.add(cur[0])
        cur[0] = cur[1] = None

    def pump(n):
        emitted = 0
        while emitted < n:
            if cur[1] is None:
                if not producers:
                    return
                cur[0], cur[1] = producers.popleft()
            try:
                next(cur[1])
                emitted += 1
            except StopIteration:
                _finish_cur()

    def pump_until(key):
        while key not in done_keys:
            if cur[1] is None:
                if not producers:
                    raise RuntimeError(f"producer underflow waiting for {key}")
                cur[0], cur[1] = producers.popleft()
            try:
                while True:
                    next(cur[1])
            except StopIteration:
                _finish_cur()

    # ---- prologue: just enough for the first scores matmul ---------------
    for _ in kT_gen(0, 0):
        pass
    for _ in qT_gen(0, 0):
        pass
    done_keys.add(("kT", 0, 0))
    done_keys.add(("qT", 0, 0))

    # initial producer order: v first (needed by cxmm), then kT/qT for the
    # upcoming head-pairs / s-tiles
    for sq in range(1, SQ):
        producers.append((("kT", 0, sq), kT_gen(0, sq)))
    for tt in range(8):
        producers.append((("v", tt), v_gen(tt)))
    producers.append((("qT", 0, 1), qT_gen(0, 1)))
    for sq in range(2):
        producers.append((("kT", 1, sq), kT_gen(1, sq)))
    for tt in range(8, TC):
        producers.append((("v", tt), v_gen(tt)))
    for sq in range(2, SQ):
        producers.append((("kT", 1, sq), kT_gen(1, sq)))
    producers.append((("qT", 0, 2), qT_gen(0, 2)))
    for sq in range(SQ):
        producers.append((("kT", 2, sq), kT_gen(2, sq)))
    producers.append((("qT", 0, 3), qT_gen(0, 3)))
    for sq in range(SQ):
        producers.append((("kT", 3, sq), kT_gen(3, sq)))

    # ---- main attention loop ---------------------------------------------
    # deferred normalize: the previous window's recip/broadcast/mul runs as
    # six small steps spread across the current window, so no single DVE
    # insertion blocks producer PSUM->SBUF copies for long.
    norm_steps = deque()

    def run_norm_step():
        if norm_steps:
            norm_steps.popleft()()

    def queue_norm(st, c, cxs):
        s0 = 512 * st
        recs = [None, None]
        bcs = [None, None]

        def recip(j):
            def fn():
                recs[j] = small.tile(
                    [1, 512], F32, tag="rec", name=f"rec{st}_{c}_{j}"
                )
                for h in range(2):
                    nc.vector.reciprocal(
                        recs[j][:, 256 * h : 256 * (h + 1)],
                        cxs[j][D : D + 1, 256 * h : 256 * (h + 1)],
                    )

            return fn

        def bcast(j):
            def fn():
                bcs[j] = small.tile(
                    [64, 512], F32, tag="tmp", name=f"bc{st}_{c}_{j}"
                )
                nc.gpsimd.partition_broadcast(bcs[j], recs[j])

            return fn

        def mul(j):
            def fn():
                nc.vector.tensor_mul(
                    ctxT_sb[64 * j : 64 * (j + 1), c, s0 : s0 + 512],
                    cxs[j][0:D, :],
                    bcs[j],
                )
                # whole s-tile normalized -> its output projection may run
                if j == 1 and c == DC - 1:
                    for si in range(4):
                        o_tile = osb.tile(
                            [128, E], F32, tag="osb", name=f"osb_{st}_{si}"
                        )
                        producers.append((("fp", st, si), fp_gen(st, si, o_tile)))

            return fn

        def both(f1, f2):
            def fn():
                f1()
                f2()

            return fn

        norm_steps.append(recip(0))
        norm_steps.append(bcast(0))
        norm_steps.append(both(mul(0), recip(1)))
        norm_steps.append(bcast(1))
        norm_steps.append(mul(1))

    for st in range(ST):
        s0 = 512 * st
        # stage the next s-tile's xq chunks well before qT(st+1) producers run
        if st + 1 < ST:
            nxt = xqp.tile([128, EC, 512], MM_DT, tag="xq", name=f"xq_st{st+1}")
            xq_tiles[st + 1] = nxt
            for e in range(EC):
                dma(
                    nxt[:, e, :],
                    xqT[128 * e : 128 * (e + 1), 512 * (st + 1) : 512 * (st + 2)],
                )
        for c in range(DC):
            if not (st == 0 and c == 0):
                pump_until(("qT", st, c))
            cx = [
                cxps.tile([DA, 512], F32, tag="cx", name=f"cx{st}_{c}_{j}")
                for j in range(2)
            ]
            first = st == 0 and c == 0
            ex_tiles = {}

            def emit_sc(t):
                if not (st == 0 and c == 0 and t < 4):
                    pump_until(("kT", c, t // 4))
                sc = scps.tile([128, 1024], F32, tag="sc")
                for j in range(2):
                    nc.tensor.matmul(
                        sc[:, 512 * j : 512 * (j + 1)],
                        lhsT=kT_sb[64 * j : 64 * (j + 1), c, 128 * t : 128 * (t + 1)],
                        rhs=qT_sb[64 * j : 64 * (j + 1), c, s0 : s0 + 512],
                        start=True,
                        stop=True,
                    )
                ex = expool.tile([128, 1024], MM_DT, tag="ex")
                nc.scalar.activation(
                    out=ex,
                    in_=sc,
                    func=mybir.ActivationFunctionType.Exp,
                    scale=float(SCALE),
                )
                ex_tiles[t] = ex

            def emit_cx(t):
                pump_until(("v", t))
                ex = ex_tiles.pop(t)
                for j in range(2):
                    nc.tensor.matmul(
                        cx[j],
                        lhsT=v_sb[:, t, 2 * c + j, :],
                        rhs=ex[:, 512 * j : 512 * (j + 1)],
                        start=(t == 0),
                        stop=(t == TC - 1),
                    )

            if first:
                # sc/exp stream ahead while v is still being produced
                for t in range(TC):
                    emit_sc(t)
                    pump(4)
                for t in range(TC):
                    emit_cx(t)
                    pump(1)
                    if t >= 3:
                        run_norm_step()
            else:
                for t in range(TC):
                    emit_sc(t)
                    emit_cx(t)
                    pump(2)
                    if t >= 3:
                        run_norm_step()

            # free the cx PSUM bank fast; the rest of the normalize is
            # deferred into the next window
            cxs = []
            for j in range(2):
                t_ = small.tile([DA, 512], F32, tag="cxs", name=f"cxs{st}_{c}_{j}")
                nc.vector.tensor_copy(t_, cx[j])
                cxs.append(t_)
            queue_norm(st, c, cxs)

        if st + 1 < ST:
            for c2 in range(DC):
                producers.append((("qT", st + 1, c2), qT_gen(st + 1, c2)))

    # ---- epilogue: last normalize + drain remaining producers -------------
    while norm_steps:
        run_norm_step()
    pump(10**9)


_BUILT = {}


def _get_nc():
    if "nc" not in _BUILT:
        _BUILT["nc"] = build_nc()
    return _BUILT["nc"]


def make_in_maps(query, key, value, Wq, Wk, Wv, Wo):
    ndt = mybir.dt.np(MM_DT)
    query = np.asarray(query, np.float32).astype(ndt)
    key = np.asarray(key, np.float32).astype(ndt)
    value = np.asarray(value, np.float32).astype(ndt)
    Wq = np.asarray(Wq, np.float32).astype(ndt)
    Wk = np.asarray(Wk, np.float32).astype(ndt)
    Wv = np.asarray(Wv, np.float32).astype(ndt)
    Wo = np.asarray(Wo, np.float32).astype(ndt)

    xqT = [np.ascontiguousarray(query[b].T) for b in range(B)]
    xkT = [np.ascontiguousarray(key[b].T) for b in range(B)]
    xvT = [np.ascontiguousarray(value[b].T) for b in range(B)]
    wqT = [np.ascontiguousarray(Wq[DH * g : DH * (g + 1), :].T) for g in range(HG)]
    wkT = [np.ascontiguousarray(Wk[DH * g : DH * (g + 1), :].T) for g in range(HG)]
    wvT = [np.ascontiguousarray(Wv[DH * g : DH * (g + 1), :].T) for g in range(HG)]
    woT = [np.ascontiguousarray(Wo[:, DH * g : DH * (g + 1)].T) for g in range(HG)]

    in_maps = []
    for core in range(NCORES):
        b, g = core // HG, core % HG
        in_maps.append(
            {
                "xqT": xqT[b],
                "xkT": xkT[b],
                "xvT": xvT[b],
                "wqT": wqT[g],
                "wkT": wkT[g],
                "wvT": wvT[g],
                "woT": woT[g],
            }
        )
    return in_maps


def assemble(core_outs):
    out = np.empty((B, S, E), np.float32)
    for b in range(B):
        out[b] = core_outs[HG * b]
        for g in range(1, HG):
            out[b] += core_outs[HG * b + g]
    return out


def kernel(query, key, value, Wq, Wk, Wv, Wo):
    from concourse.bass_utils import run_bass_kernel_spmd

    nc = _get_nc()
    in_maps = make_in_maps(query, key, value, Wq, Wk, Wv, Wo)
    res = run_bass_kernel_spmd(nc, in_maps, list(range(NCORES)))
    return assemble([r["out"] for r in res.results])
